# revision 3
# baseline (speedup 1.0000x reference)
"""Trainium2 Bass kernel for the batched dynamic bicycle model.

Data-parallel across 8 NeuronCores: each core integrates B/8 rows for 10
Euler substeps. Layout: rows interleaved in DRAM as (R, 9) f32; each SBUF
tile holds 128*N rows as [128, 9N] with state variable j at free-dim
stride-9 offset j. All state updates happen in place on that tile, which
is then DMA'd back out. The a/delta first-order lags are tracked as the
scaled planes ha = h*a and cd = CF*delta (linear recurrences), and the
trig args are range-wrapped into [-pi, pi] before the ACT-engine Sin
table (cos(x) = sin(x + pi/2)).

Per-substep arithmetic runs as fused custom DVE ops (registered below)
to minimize DVE passes — the kernel is Vector-engine-bound.
"""

import numpy as np

import concourse.bacc as bacc
import concourse.bass as bass
import concourse.mybir as mybir
import concourse.tile as tile
import concourse.dve_ops as dve_ops
from concourse.dve_spec import (
    Spec, Src0, Src1, C0, C1, C2, Zero, relu, maxx, minn, lower,
    _has_src1 as has_src1,
)
from concourse.dve_uop import DveOpSpec
from concourse.bass_utils import run_bass_kernel_spmd

AluOp = mybir.AluOpType
FN = mybir.ActivationFunctionType
F32 = mybir.dt.float32

PI = 3.141592653589793
M = 1500.0
IZ = 2250.0
LF = 1.2
LR = 1.6
CF = 80000.0
CR = 80000.0
TAU = 0.1
MAX_STEER = 30.0 * PI / 180.0
MAX_ACCEL = 3.0
MIN_ACCEL = -6.0
MU = 0.9
G = 9.81
L = LF + LR
FYF = MU * M * G * (LR / L)
FYR = MU * M * G * (LF / L)
VMIN = 20.0 / 3.6

HD = float(np.float32(0.01))               # f32 substep h, as a double
K = float(np.float32(1.0 - HD / TAU))      # f32 per-substep lag decay
NSUB = 10

B_FULL = 4194304
NCORES = 8
R_CORE = B_FULL // NCORES                  # 524288 rows per core
P = 128


# ---------------------------------------------------------------- custom ops
def _register(name, spec):
    for op in dve_ops.OPS:
        if op.name == name:
            return op
    opcode = dve_ops._CUSTOM_DVE_ROW_BASE + len(dve_ops.OPS)
    shas = {}
    for ver in ("v3", "v4"):
        s = DveOpSpec(
            name=name, opcode=opcode, uops=lower(spec, ver=ver),
            rd1_en=has_src1(spec),
        )
        shas[ver] = s.sha(ver)
    op = dve_ops.DveOp(name, spec, subdim=False, uops_sha=shas)
    dve_ops.OPS.append(op)
    dve_ops._SUB_OPCODE_FOR_NAME[name] = opcode
    dve_ops.CUSTOM_DVE_SPECS[name] = spec
    return op


# out = s0*in0 + s1*in1
LINEAR2 = _register(
    "ANT_BM_LINEAR2",
    Spec(
        body=Src0 * C0 + Src1 * C1,
        reference=lambda in0, in1, s0, s1, imm2: (in0 * s0 + in1 * s1).astype(
            np.float32
        ),
    ),
)
# out = clip(s0*in0 + s1*in1, imm2, -imm2)   (imm2 = the NEGATIVE bound)
CLIP_LIN2 = _register(
    "ANT_BM_CLIP_LIN2",
    Spec(
        body=minn(maxx(Src0 * C0 + Src1 * C1, C2), Zero - C2),
        reference=lambda in0, in1, s0, s1, imm2: np.minimum(
            np.maximum(in0 * s0 + in1 * s1, imm2), -imm2
        ).astype(np.float32),
    ),
)
# y = in0 + in1 + s0; out = y + imm2*((y < -s1) - (y > s1))
_y = Src0 + Src1 + C0
ADD2_WRAP = _register(
    "ANT_BM_ADD2_WRAP",
    Spec(
        body=_y + C2 * ((_y < Zero - C1) - (_y > C1)),
        reference=lambda in0, in1, s0, s1, imm2: (
            (in0 + in1 + s0)
            + imm2
            * (
                ((in0 + in1 + s0) < -s1).astype(np.float32)
                - ((in0 + in1 + s0) > s1).astype(np.float32)
            )
        ).astype(np.float32),
    ),
)
# out = relu(in0 + in1)
ADD_RELU = _register(
    "ANT_BM_ADD_RELU",
    Spec(
        body=relu(Src0 + Src1),
        reference=lambda in0, in1, s0, s1, imm2: np.maximum(in0 + in1, 0).astype(
            np.float32
        ),
    ),
)
# out = s0 * in0 * in1
MUL2S = _register(
    "ANT_BM_MUL2S",
    Spec(
        body=Src0 * Src1 * C0,
        reference=lambda in0, in1, s0, s1, imm2: (in0 * in1 * s0).astype(np.float32),
    ),
)


# ---------------------------------------------------------------- kernel body
def emit_bicycle(tc, st_ap, ac_ap, out_ap, R, N):
    """Emit the whole per-core program: R rows, free-dim tile width N."""
    nc = tc.nc
    rows_per_tile = P * N
    T = R // rows_per_tile
    assert T * rows_per_tile == R

    st_t = st_ap.rearrange("(t p n) v -> t p (n v)", t=T, p=P, n=N)
    ac_t = ac_ap.rearrange("(t p n) v -> t p (n v)", t=T, p=P, n=N)
    out_t = out_ap.rearrange("(t p n) v -> t p (n v)", t=T, p=P, n=N)

    ts_ = nc.vector.tensor_scalar
    stt = nc.vector.scalar_tensor_tensor
    tt = nc.vector.tensor_tensor
    cdve = nc.vector._custom_dve

    with (
        tc.tile_pool(name="io", bufs=2) as io,
        tc.tile_pool(name="pl", bufs=2) as pl,
        tc.tile_pool(name="tp", bufs=2) as tp,
    ):
        for t in range(T):
            big = io.tile([P, 9 * N], F32, tag="big")
            act = io.tile([P, 2 * N], F32, tag="act")
            nc.sync.dma_start(big[:], st_t[t])
            nc.sync.dma_start(act[:], ac_t[t])

            bigv = big.rearrange("p (n v) -> p v n", v=9)
            sx, sy, spsi, sv = bigv[:, 0], bigv[:, 1], bigv[:, 2], bigv[:, 3]
            sa, sd, sbeta, sr, sdref = (
                bigv[:, 4], bigv[:, 5], bigv[:, 6], bigv[:, 7], bigv[:, 8],
            )
            actv = act.rearrange("p (n v) -> p v n", v=2)

            cd = pl.tile([P, N], F32, tag="cd")
            cdc = pl.tile([P, N], F32, tag="cdc")
            ha = pl.tile([P, N], F32, tag="ha")
            hac = pl.tile([P, N], F32, tag="hac")
            b = [tp.tile([P, N], F32, tag=f"b{i}", name=f"b{i}") for i in range(8)]
            b0, b1, b2, b3, b4, b5, b6, b7 = b

            # ---- PRE: reset clamps, action clip, lag planes ----
            ts_(b0[:], actv[:, 0], MIN_ACCEL, MAX_ACCEL, AluOp.max, AluOp.min)
            ts_(sdref, actv[:, 1], -MAX_STEER, MAX_STEER, AluOp.max, AluOp.min)
            ts_(sv, sv, 0.0, None, AluOp.max)
            ts_(sd, sd, -MAX_STEER, MAX_STEER, AluOp.max, AluOp.min)
            ts_(cd[:], sd, CF, None, AluOp.mult)
            ts_(cdc[:], sdref, CF * (1.0 - K), None, AluOp.mult)
            ts_(ha[:], sa, HD, None, AluOp.mult)
            ts_(hac[:], b0[:], HD * (1.0 - K), None, AluOp.mult)

            # ---- 10 Euler substeps, all f32 ----
            for n in range(NSUB):
                # trig args from OLD psi, beta (wrap into [-pi, pi])
                cdve(ADD2_WRAP, out=b1[:], in0=spsi, in1=sbeta,
                     s0=0.0, s1=PI, imm2=2 * PI)                    # ws
                cdve(ADD2_WRAP, out=b2[:], in0=spsi, in1=sbeta,
                     s0=PI / 2, s1=PI, imm2=2 * PI)                 # wc
                nc.scalar.activation(b3[:], b2[:], FN.Sin)          # c = cos
                nc.scalar.activation(b4[:], b1[:], FN.Sin)          # sn = sin
                # slip / forces
                ts_(b5[:], sv, VMIN, None, AluOp.max)               # ve
                nc.vector.reciprocal_approx_fast(b5[:], b5[:])      # iv
                tt(b0[:], sr, b5[:], AluOp.mult)                    # rv
                cdve(LINEAR2, out=b1[:], in0=cd[:], in1=sbeta,
                     s0=1.0, s1=-CF)                                # u
                cdve(CLIP_LIN2, out=b6[:], in0=b1[:], in1=b0[:],
                     s0=1.0, s1=-CF * LF, imm2=-FYF)                # Ff
                cdve(CLIP_LIN2, out=b7[:], in0=b0[:], in1=sbeta,
                     s0=CR * LR, s1=-CF, imm2=-FYR)                 # Fr
                tt(b1[:], b6[:], b7[:], AluOp.add)                  # s
                tt(b2[:], b1[:], b5[:], AluOp.mult)                 # fb
                cdve(LINEAR2, out=b5[:], in0=b2[:], in1=sr,
                     s0=1.0 / M, s1=-1.0)                           # w
                cdve(LINEAR2, out=b0[:], in0=b6[:], in1=b7[:],
                     s0=LF / LR, s1=-1.0)                           # q
                cdve(MUL2S, out=b1[:], in0=sv, in1=b3[:], s0=HD)    # mx
                cdve(MUL2S, out=b2[:], in0=sv, in1=b4[:], s0=HD)    # my
                tt(sx, sx, b1[:], AluOp.add)                        # x +=
                tt(sy, sy, b2[:], AluOp.add)                        # y +=
                cdve(ADD_RELU, out=sv, in0=sv, in1=ha[:])           # v'
                stt(spsi, sr, HD, spsi, AluOp.mult, AluOp.add)      # psi +=
                stt(sbeta, b5[:], HD, sbeta, AluOp.mult, AluOp.add)  # beta +=
                stt(sr, b0[:], LR * HD / IZ, sr, AluOp.mult, AluOp.add)  # r +=
                stt(cd[:], cd[:], K, cdc[:], AluOp.mult, AluOp.add)  # cd lag
                stt(ha[:], ha[:], K, hac[:], AluOp.mult, AluOp.add)  # ha lag

            # ---- POST: recover a, delta from lag planes; store ----
            ts_(sd, cd[:], 1.0 / CF, None, AluOp.mult)
            ts_(sa, ha[:], 1.0 / HD, None, AluOp.mult)
            nc.sync.dma_start(out_t[t], big[:])


def build_nc(R=R_CORE, N=1024):
    nc = bacc.Bacc("TRN2", target_bir_lowering=False, debug=False)
    st = nc.dram_tensor("init_state", [R, 9], F32, kind="ExternalInput")
    ac = nc.dram_tensor("action", [R, 2], F32, kind="ExternalInput")
    out = nc.dram_tensor("out", [R, 9], F32, kind="ExternalOutput")
    with tile.TileContext(nc) as tc:
        emit_bicycle(tc, st.ap(), ac.ap(), out.ap(), R, N)
    nc.compile()
    return nc


_NC_CACHE = {}

# Optional bench knobs (used by the local test harness only).
BENCH = {"trace": False}
LAST_RESULTS = None


def kernel(init_state: np.ndarray, action: np.ndarray) -> np.ndarray:
    global LAST_RESULTS
    assert init_state.shape == (B_FULL, 9) and action.shape == (B_FULL, 2)
    key = "full"
    if key not in _NC_CACHE:
        _NC_CACHE[key] = build_nc()
    nc = _NC_CACHE[key]
    st = np.ascontiguousarray(init_state, dtype=np.float32)
    ac = np.ascontiguousarray(action, dtype=np.float32)
    in_maps = [
        {
            "init_state": st[i * R_CORE : (i + 1) * R_CORE],
            "action": ac[i * R_CORE : (i + 1) * R_CORE],
        }
        for i in range(NCORES)
    ]
    res = run_bass_kernel_spmd(
        nc, in_maps, core_ids=list(range(NCORES)), trace=BENCH["trace"]
    )
    LAST_RESULTS = res
    return np.concatenate([res.results[i]["out"] for i in range(NCORES)], axis=0)


# revision 5
# speedup vs baseline: 1.0106x; 1.0106x over previous
"""Trainium2 Bass kernel for the batched dynamic bicycle model.

Data-parallel across 8 NeuronCores: each core integrates B/8 rows for 10
Euler substeps. Layout: rows interleaved in DRAM as (R, 9) f32; each SBUF
tile holds 128*N rows as [128, 9N] with state variable j at free-dim
stride-9 offset j. All state updates happen in place on that tile, which
is then DMA'd back out. The a/delta first-order lags are tracked as the
scaled planes ha = h*a and cd = CF*delta (linear recurrences), and the
trig args are range-wrapped into [-pi, pi] before the ACT-engine Sin
table (cos(x) = sin(x + pi/2)).

Per-substep arithmetic runs as fused custom DVE ops (registered below)
to minimize DVE passes — the kernel is Vector-engine-bound.
"""

import numpy as np

import concourse.bacc as bacc
import concourse.bass as bass
import concourse.mybir as mybir
import concourse.tile as tile
import concourse.dve_ops as dve_ops
from concourse.dve_spec import (
    Spec, Src0, Src1, C0, C1, C2, Zero, relu, maxx, minn, lower,
    _has_src1 as has_src1,
)
from concourse.dve_uop import DveOpSpec
from concourse.bass_utils import run_bass_kernel_spmd

AluOp = mybir.AluOpType
FN = mybir.ActivationFunctionType
F32 = mybir.dt.float32

PI = 3.141592653589793
M = 1500.0
IZ = 2250.0
LF = 1.2
LR = 1.6
CF = 80000.0
CR = 80000.0
TAU = 0.1
MAX_STEER = 30.0 * PI / 180.0
MAX_ACCEL = 3.0
MIN_ACCEL = -6.0
MU = 0.9
G = 9.81
L = LF + LR
FYF = MU * M * G * (LR / L)
FYR = MU * M * G * (LF / L)
VMIN = 20.0 / 3.6

HD = float(np.float32(0.01))               # f32 substep h, as a double
K = float(np.float32(1.0 - HD / TAU))      # f32 per-substep lag decay
NSUB = 10

B_FULL = 4194304
NCORES = 8
R_CORE = B_FULL // NCORES                  # 524288 rows per core
P = 128


# ---------------------------------------------------------------- custom ops
def _register(name, spec):
    for op in dve_ops.OPS:
        if op.name == name:
            return op
    opcode = dve_ops._CUSTOM_DVE_ROW_BASE + len(dve_ops.OPS)
    shas = {}
    for ver in ("v3", "v4"):
        s = DveOpSpec(
            name=name, opcode=opcode, uops=lower(spec, ver=ver),
            rd1_en=has_src1(spec),
        )
        shas[ver] = s.sha(ver)
    op = dve_ops.DveOp(name, spec, subdim=False, uops_sha=shas)
    dve_ops.OPS.append(op)
    dve_ops._SUB_OPCODE_FOR_NAME[name] = opcode
    dve_ops.CUSTOM_DVE_SPECS[name] = spec
    return op


# out = s0*in0 + s1*in1
LINEAR2 = _register(
    "ANT_BM_LINEAR2",
    Spec(
        body=Src0 * C0 + Src1 * C1,
        reference=lambda in0, in1, s0, s1, imm2: (in0 * s0 + in1 * s1).astype(
            np.float32
        ),
    ),
)
# out = clip(s0*in0 + s1*in1, imm2, -imm2)   (imm2 = the NEGATIVE bound)
CLIP_LIN2 = _register(
    "ANT_BM_CLIP_LIN2",
    Spec(
        body=minn(maxx(Src0 * C0 + Src1 * C1, C2), Zero - C2),
        reference=lambda in0, in1, s0, s1, imm2: np.minimum(
            np.maximum(in0 * s0 + in1 * s1, imm2), -imm2
        ).astype(np.float32),
    ),
)
# y = in0 + in1 + s0; out = y + imm2*((y < -s1) - (y > s1))
_y = Src0 + Src1 + C0
ADD2_WRAP = _register(
    "ANT_BM_ADD2_WRAP",
    Spec(
        body=_y + C2 * ((_y < Zero - C1) - (_y > C1)),
        reference=lambda in0, in1, s0, s1, imm2: (
            (in0 + in1 + s0)
            + imm2
            * (
                ((in0 + in1 + s0) < -s1).astype(np.float32)
                - ((in0 + in1 + s0) > s1).astype(np.float32)
            )
        ).astype(np.float32),
    ),
)
# out = relu(in0 + in1)
ADD_RELU = _register(
    "ANT_BM_ADD_RELU",
    Spec(
        body=relu(Src0 + Src1),
        reference=lambda in0, in1, s0, s1, imm2: np.maximum(in0 + in1, 0).astype(
            np.float32
        ),
    ),
)
# out = s0 * in0 * in1
MUL2S = _register(
    "ANT_BM_MUL2S",
    Spec(
        body=Src0 * Src1 * C0,
        reference=lambda in0, in1, s0, s1, imm2: (in0 * in1 * s0).astype(np.float32),
    ),
)
# out = clip(in0, s0, s1) * imm2   (single-src)
CLIP_SCALE = _register(
    "ANT_BM_CLIP_SCALE",
    Spec(
        body=minn(maxx(Src0, C0), C1) * C2,
        reference=lambda in0, in1, s0, s1, imm2: (
            np.minimum(np.maximum(in0, s0), s1) * imm2
        ).astype(np.float32),
    ),
)


# ---------------------------------------------------------------- kernel body
def emit_bicycle(tc, st_ap, ac_ap, out_ap, R, N):
    """Emit the whole per-core program: R rows, free-dim tile width N."""
    nc = tc.nc
    rows_per_tile = P * N
    T = R // rows_per_tile
    assert T * rows_per_tile == R

    st_t = st_ap.rearrange("(t p n) v -> t p (n v)", t=T, p=P, n=N)
    ac_t = ac_ap.rearrange("(t p n) v -> t p (n v)", t=T, p=P, n=N)
    out_t = out_ap.rearrange("(t p n) v -> t p (n v)", t=T, p=P, n=N)

    ts_ = nc.vector.tensor_scalar
    stt = nc.vector.scalar_tensor_tensor
    tt = nc.vector.tensor_tensor
    cdve = nc.vector._custom_dve

    with (
        tc.tile_pool(name="io", bufs=2) as io,
        tc.tile_pool(name="pl", bufs=2) as pl,
        tc.tile_pool(name="tp", bufs=2) as tp,
    ):
        for t in range(T):
            big = io.tile([P, 9 * N], F32, tag="big")
            act = io.tile([P, 2 * N], F32, tag="act")
            nc.sync.dma_start(big[:], st_t[t])
            nc.sync.dma_start(act[:], ac_t[t])

            bigv = big.rearrange("p (n v) -> p v n", v=9)
            sx, sy, spsi, sv = bigv[:, 0], bigv[:, 1], bigv[:, 2], bigv[:, 3]
            sa, sd, sbeta, sr, sdref = (
                bigv[:, 4], bigv[:, 5], bigv[:, 6], bigv[:, 7], bigv[:, 8],
            )
            actv = act.rearrange("p (n v) -> p v n", v=2)

            cd = pl.tile([P, N], F32, tag="cd")
            cdc = pl.tile([P, N], F32, tag="cdc")
            ha = pl.tile([P, N], F32, tag="ha")
            hac = pl.tile([P, N], F32, tag="hac")
            vp = pl.tile([P, N], F32, tag="vp")
            b = [tp.tile([P, N], F32, tag=f"b{i}", name=f"b{i}") for i in range(8)]
            b0, b1, b2, b3, b4, b5, b6, b7 = b

            # ---- PRE: reset clamps, action clip, lag planes ----
            # (cheap single-tensor scale/copy work rides the idle ACT engine)
            cdve(CLIP_SCALE, out=hac[:], in0=actv[:, 0],
                 s0=MIN_ACCEL, s1=MAX_ACCEL, imm2=HD * (1.0 - K))
            ts_(sdref, actv[:, 1], -MAX_STEER, MAX_STEER, AluOp.max, AluOp.min)
            nc.scalar.activation(cdc[:], sdref, FN.Copy, scale=CF * (1.0 - K))
            nc.scalar.activation(vp[:], sv, FN.Relu)            # v = max(v0, 0)
            cdve(CLIP_SCALE, out=cd[:], in0=sd,
                 s0=-MAX_STEER, s1=MAX_STEER, imm2=CF)
            nc.scalar.activation(ha[:], sa, FN.Copy, scale=HD)

            # ---- 10 Euler substeps, all f32 ----
            for n in range(NSUB):
                # trig args from OLD psi, beta (wrap into [-pi, pi])
                cdve(ADD2_WRAP, out=b1[:], in0=spsi, in1=sbeta,
                     s0=0.0, s1=PI, imm2=2 * PI)                    # ws
                cdve(ADD2_WRAP, out=b2[:], in0=spsi, in1=sbeta,
                     s0=PI / 2, s1=PI, imm2=2 * PI)                 # wc
                nc.scalar.activation(b3[:], b2[:], FN.Sin)          # c = cos
                nc.scalar.activation(b4[:], b1[:], FN.Sin)          # sn = sin
                # slip / forces
                ts_(b5[:], vp[:], VMIN, None, AluOp.max)            # ve (2x mode)
                nc.vector.reciprocal_approx_fast(b5[:], b5[:])      # iv
                tt(b0[:], sr, b5[:], AluOp.mult)                    # rv
                cdve(LINEAR2, out=b1[:], in0=cd[:], in1=sbeta,
                     s0=1.0, s1=-CF)                                # u
                cdve(CLIP_LIN2, out=b6[:], in0=b1[:], in1=b0[:],
                     s0=1.0, s1=-CF * LF, imm2=-FYF)                # Ff
                cdve(CLIP_LIN2, out=b7[:], in0=b0[:], in1=sbeta,
                     s0=CR * LR, s1=-CF, imm2=-FYR)                 # Fr
                tt(b1[:], b6[:], b7[:], AluOp.add)                  # s
                tt(b2[:], b1[:], b5[:], AluOp.mult)                 # fb
                cdve(LINEAR2, out=b5[:], in0=b2[:], in1=sr,
                     s0=1.0 / M, s1=-1.0)                           # w
                cdve(LINEAR2, out=b0[:], in0=b6[:], in1=b7[:],
                     s0=LF / LR, s1=-1.0)                           # q
                cdve(MUL2S, out=b1[:], in0=vp[:], in1=b3[:], s0=HD)  # mx
                cdve(MUL2S, out=b2[:], in0=vp[:], in1=b4[:], s0=HD)  # my
                tt(sx, sx, b1[:], AluOp.add)                        # x +=
                tt(sy, sy, b2[:], AluOp.add)                        # y +=
                cdve(ADD_RELU, out=vp[:], in0=vp[:], in1=ha[:])     # v'
                stt(spsi, sr, HD, spsi, AluOp.mult, AluOp.add)      # psi +=
                stt(sbeta, b5[:], HD, sbeta, AluOp.mult, AluOp.add)  # beta +=
                stt(sr, b0[:], LR * HD / IZ, sr, AluOp.mult, AluOp.add)  # r +=
                stt(cd[:], cd[:], K, cdc[:], AluOp.mult, AluOp.add)  # cd lag
                stt(ha[:], ha[:], K, hac[:], AluOp.mult, AluOp.add)  # ha lag

            # ---- POST: recover outputs (on ACT; DVE stays on substeps) ----
            nc.scalar.activation(sd, cd[:], FN.Copy, scale=1.0 / CF)
            nc.scalar.activation(sa, ha[:], FN.Copy, scale=1.0 / HD)
            nc.scalar.activation(sv, vp[:], FN.Copy, scale=1.0)
            nc.sync.dma_start(out_t[t], big[:])


def build_nc(R=R_CORE, N=1024):
    nc = bacc.Bacc("TRN2", target_bir_lowering=False, debug=False)
    st = nc.dram_tensor("init_state", [R, 9], F32, kind="ExternalInput")
    ac = nc.dram_tensor("action", [R, 2], F32, kind="ExternalInput")
    out = nc.dram_tensor("out", [R, 9], F32, kind="ExternalOutput")
    with tile.TileContext(nc) as tc:
        emit_bicycle(tc, st.ap(), ac.ap(), out.ap(), R, N)
    nc.compile()
    return nc


_NC_CACHE = {}

# Optional bench knobs (used by the local test harness only).
BENCH = {"trace": False}
LAST_RESULTS = None


def kernel(init_state: np.ndarray, action: np.ndarray) -> np.ndarray:
    global LAST_RESULTS
    assert init_state.shape == (B_FULL, 9) and action.shape == (B_FULL, 2)
    key = "full"
    if key not in _NC_CACHE:
        _NC_CACHE[key] = build_nc()
    nc = _NC_CACHE[key]
    st = np.ascontiguousarray(init_state, dtype=np.float32)
    ac = np.ascontiguousarray(action, dtype=np.float32)
    in_maps = [
        {
            "init_state": st[i * R_CORE : (i + 1) * R_CORE],
            "action": ac[i * R_CORE : (i + 1) * R_CORE],
        }
        for i in range(NCORES)
    ]
    res = run_bass_kernel_spmd(
        nc, in_maps, core_ids=list(range(NCORES)), trace=BENCH["trace"]
    )
    LAST_RESULTS = res
    return np.concatenate([res.results[i]["out"] for i in range(NCORES)], axis=0)
